# revision 26
# baseline (speedup 1.0000x reference)
"""Trainium2 Bass kernel for nn_BivariateNormalAttention.

Self-contained: takes FULL inputs (B=16), shards batch across 8 NeuronCores
(2 images/core), runs a Bass/Tile kernel per core, gathers [16,8,56,56].

Pipeline per image:
  conv3x3(512->256)+BN+ReLU -> conv3x3(256->256)+BN+ReLU -> avgpool16 (7x7)
  -> conv3x3(256->128)+BN+ReLU -> conv3x3(128->128)+BN+ReLU -> avgpool3s2 (3x3)
  -> conv3x3(128->64)+BN+ReLU -> fc(576->128) -> bivariate-normal attention maps.

Convs 1-2 (99.9% of FLOPs) run as 9-tap shifted matmuls in fp8-e4m3 with
perf_mode=DoubleRow (two 128-deep K blocks per instruction at 0.5 cyc/row).
Weight loads amortize over groups of 4 row-strips. The fp8 weight-quantization
bias (which couples to the nonzero mean of post-ReLU activations) is removed
by a per-image bias correction: corr = (sum_taps dw2) @ mean(h1), computed on
device from a running sum of conv1 activations. Convs 3-5 / fc / attention run
in fp32; the attention tail batches both images on 64 partitions.
"""
import sys
import numpy as np

for _p in ("/opt/trn_rl_repo", "/root/.axon_site/_ro/trn_rl_repo"):
    if _p not in sys.path:
        sys.path.append(_p)

import concourse.bacc as bacc
import concourse.mybir as mybir
import concourse.tile as tile
from concourse.bass_utils import run_bass_kernel_spmd

F32 = mybir.dt.float32
F8 = mybir.dt.float8e4
DR = mybir.MatmulPerfMode.DoubleRow
DRS = mybir.MatmulPerfMode.DoubleRowSwInterleave

B, C, H, W = 16, 512, 112, 112
OUT, GMM = 8, 4
NCORE = 8
IMG = B // NCORE                 # 2 images per core
HP, WP = H + 2, W + 2            # 114
FLAT = HP * WP                   # 12996
RS = 4                           # conv strip rows
NSTRIP = H // RS                 # 28
GS = 4                           # strips per weight-load group
NGRP = NSTRIP // GS              # 7
NFREE = RS * WP                  # 456
XL = 688                         # per-ci lane length (6*WP+2=686 padded to 16)
H2 = W2 = H // 2                 # 56
SIG2 = float(H) / 2.0            # sigma = 56
LOGR = float(np.log(3.0))


def build_nc(conv_dtype="fp8", r_loop=None, ldma="whole4", xbufs=8, pbufs=8,
             cbufs=2, phases="abc", mm_mode="swi", gs=4, corr=True,
             sorder="ki", skel=None, nblk=4, obufs=12):
    """Bass program for one core processing IMG images."""
    nc = bacc.Bacc("TRN2", target_bir_lowering=False, debug=False)

    MM = DRS if mm_mode == "swi" else DR
    x = nc.dram_tensor("x", [IMG, 4, 128, HP, WP], F8, kind="ExternalInput")
    if mm_mode == "swi":
        w1t = nc.dram_tensor("w1t", [128, 9, 2, 2, 256], F8,
                             kind="ExternalInput")
        w2t = nc.dram_tensor("w2t", [128, 9, 2, 256], F8,
                             kind="ExternalInput")
    else:
        w1t = nc.dram_tensor("w1t", [128, 9, 4, 256], F8,
                             kind="ExternalInput")
        w2t = nc.dram_tensor("w2t", [128, 9, 2, 256], F8,
                             kind="ExternalInput")
    w3t = nc.dram_tensor("w3t", [128, 9, 2, 128], F32, kind="ExternalInput")
    w4t = nc.dram_tensor("w4t", [128, 9, 128], F32, kind="ExternalInput")
    w5t = nc.dram_tensor("w5t", [128, 9, 64], F32, kind="ExternalInput")
    wfct = nc.dram_tensor("wfct", [64, 9, 128], F32, kind="ExternalInput")
    b1d = nc.dram_tensor("b1d", [128, 2], F32, kind="ExternalInput")
    b2d = nc.dram_tensor("b2d", [128, 2], F32, kind="ExternalInput")
    b3d = nc.dram_tensor("b3d", [128, 1], F32, kind="ExternalInput")
    b4d = nc.dram_tensor("b4d", [128, 1], F32, kind="ExternalInput")
    b5d = nc.dram_tensor("b5d", [64, 1], F32, kind="ExternalInput")
    selpd = nc.dram_tensor("selpd", [128, 128], F32, kind="ExternalInput")
    selgd = nc.dram_tensor("selgd", [64, 16], F32, kind="ExternalInput")
    negiod = nc.dram_tensor("negiod", [64, 56], F32, kind="ExternalInput")
    cstd = nc.dram_tensor("cstd", [64, 1], F32, kind="ExternalInput")  # -ln3
    selSd = nc.dram_tensor("selSd", [128, 2, 2, 128], F32,
                           kind="ExternalInput")  # conv2 fp8 mean correction

    out = nc.dram_tensor("out", [IMG, OUT, H2, W2], F32, kind="ExternalOutput")

    ldma_eng = {"gpsimd": nc.gpsimd,
                "sync": nc.sync}.get(ldma, nc.sync)

    with tile.TileContext(nc) as tc:
        with tc.tile_pool(name="persist", bufs=1) as pp:
            b1 = pp.tile([128, 2], F32)
            b2 = pp.tile([128, 2], F32)
            selS = pp.tile([128, 2, 2, 128], F32)
            if mm_mode == "swi":
                w1 = pp.tile([128, 9, 2, 2, 256], F8, name="w1")
            else:
                w1 = pp.tile([128, 9, 4, 256], F8, name="w1")
            w2 = pp.tile([128, 9, 2, 256], F8, name="w2")
            w3 = pp.tile([128, 9, 2, 128], F32, name="w3")
            w4 = pp.tile([128, 9, 128], F32, name="w4")
            w5 = pp.tile([128, 9, 64], F32, name="w5")
            wfc = pp.tile([64, 9, 128], F32, name="wfc")
            b3 = pp.tile([128, 1], F32, name="b3")
            b4 = pp.tile([128, 1], F32, name="b4")
            b5 = pp.tile([64, 1], F32, name="b5")
            selp = pp.tile([128, 128], F32, name="selp")
            selg = pp.tile([64, 16], F32, name="selg")
            negio = pp.tile([64, 56], F32, name="negio")
            cst = pp.tile([64, 1], F32, name="cst")
            # constants loaded ONCE per launch (outside the timing loop)
            nc.gpsimd.dma_start(w1[:], w1t[:])
            nc.gpsimd.dma_start(w2[:], w2t[:])
            nc.gpsimd.dma_start(b1[:], b1d[:])
            nc.gpsimd.dma_start(b2[:], b2d[:])
            nc.gpsimd.dma_start(selS[:], selSd[:])
            for tdst, tsrc in ((w3, w3t), (w4, w4t), (w5, w5t),
                               (wfc, wfct), (b3, b3d), (b4, b4d),
                               (b5, b5d), (selp, selpd), (selg, selgd),
                               (negio, negiod), (cst, cstd)):
                nc.gpsimd.dma_start(tdst[:], tsrc[:])
            # per-(img, chunk) sums of conv1 activations / pool accums
            hsum = [[pp.tile([128, 1], F32, name=f"hs{i}_{c}",
                             tag=f"hs{i}_{c}")
                     for c in range(2)] for i in range(IMG)]
            b2eff = [pp.tile([128, 2], F32, name=f"b2e{i}",
                             tag=f"b2e{i}")
                     for i in range(IMG)]
            pacc = [[pp.tile([128, 49], F32, name=f"pacc{i}_{c}",
                             tag=f"pacc{i}_{c}")
                     for c in range(2)] for i in range(IMG)]

            def emit_body():

                # ---- phases A+B per image, x and conv1 output resident in
                # SBUF (c1 intermediate never round-trips through DRAM)
                XIL = 13008  # FLAT=12996 padded to a 16B multiple
                with (
                    tc.tile_pool(name="xpi", bufs=2) as xpi,
                    tc.tile_pool(name="cpi", bufs=2) as cpi,
                    tc.tile_pool(name="oa", bufs=4) as oa,
                    tc.tile_pool(name="ob", bufs=6) as ob,
                    tc.tile_pool(name="ps8", bufs=pbufs, space="PSUM") as ps8,
                ):
                    ngrp = NSTRIP // gs
                    for img in range(IMG):
                        xi = xpi.tile([128, 4, XIL], F8, tag="xi")
                        ci = cpi.tile([128, 2, XIL], F8, tag="ci")
                        xflat = x[img].rearrange("c p a b -> p c (a b)")
                        civ = ci[:, :, 0:FLAT].rearrange(
                            "p c (a b) -> p c a b", b=WP)
                        nc.vector.memset(xi[:, :, FLAT:XIL], 0.0)
                        nc.vector.memset(ci[:, :, FLAT:XIL], 0.0)
                        nc.vector.memset(civ[:, :, 0, :], 0.0)
                        nc.vector.memset(civ[:, :, HP - 1, :], 0.0)
                        nc.vector.memset(civ[:, :, :, 0], 0.0)
                        nc.vector.memset(civ[:, :, :, WP - 1], 0.0)
                        hs28 = [oa.tile([128, NSTRIP], F32, name=f"hs28_{c}",
                                        tag=f"hs28_{c}") for c in range(2)]

                        # ---------------- conv1 (fp8 DoubleRow) -------------
                        if "a" in phases:
                            if ldma.startswith("whole"):
                                spec = ldma[5:]
                                par = spec.endswith("q")
                                nchunk = int(spec.rstrip("q") or 1)
                                qs = ([nc.sync, nc.gpsimd, nc.scalar]
                                      if par else [nc.sync])
                                cuts = [0] + [
                                    ((FLAT * (i + 1) // nchunk) // 16) * 16
                                    for i in range(nchunk - 1)] + [FLAT]
                                for i in range(nchunk):
                                    qs[i % len(qs)].dma_start(
                                        xi[:, :, cuts[i]:cuts[i + 1]],
                                        xflat[:, :, cuts[i]:cuts[i + 1]])
                            for g in range(ngrp):
                                if ldma.startswith("whole"):
                                    pass
                                elif g == 0:
                                    lo, hi = 0, (4 * gs + 2) * WP
                                    ldma_eng.dma_start(xi[:, :, lo:hi],
                                                       xflat[:, :, lo:hi])
                                else:
                                    lo = (4 * gs * g + 2) * WP
                                    hi = min(FLAT, (4 * gs * (g + 1) + 2) * WP)
                                    ldma_eng.dma_start(xi[:, :, lo:hi],
                                                       xflat[:, :, lo:hi])
                                for co in range(2):
                                    ps = [ps8.tile([128, NFREE], F32,
                                                   name="psa", tag="psa")
                                          for _ in range(gs)]
                                    for k in range(18):
                                        p, t = divmod(k, 9)
                                        if mm_mode == "swi":
                                            wap = w1[:, t, p, co].rearrange(
                                                "q (a b) -> q a b", b=128)
                                        else:
                                            wap = w1[:, t, 2 * p:2 * p + 2,
                                                     co * 128:(co + 1) * 128]
                                        sh = (t // 3) * WP + t % 3
                                        for s in range(gs):
                                            off = RS * (g * gs + s) * WP + sh
                                            nc.tensor.matmul(
                                                ps[s][:], wap,
                                                xi[:, 2 * p:2 * p + 2,
                                                   off:off + NFREE],
                                                start=(k == 0),
                                                stop=(k == 17),
                                                perf_mode=MM)
                                    for s in range(gs):
                                        sg = g * gs + s
                                        base = (1 + RS * sg) * WP
                                        if skel == "mmonly":
                                            ob_ = oa.tile([128, NFREE], F32,
                                                          tag="ob_")
                                            nc.vector.tensor_copy(ob_[:],
                                                                  ps[s][:])
                                            continue
                                        nc.scalar.activation(
                                            ci[:, co, base:base + NFREE]
                                            .rearrange("p (a b) -> p a b",
                                                       b=WP)[:, :, 1:113],
                                            ps[s][:].rearrange(
                                                "p (a b) -> p a b",
                                                b=WP)[:, :, 0:112],
                                            mybir.ActivationFunctionType.Relu,
                                            bias=b1[:, co:co + 1],
                                            accum_out=(hs28[co][:, sg:sg + 1]
                                                       if corr else None))

                        if "b" not in phases:
                            continue
                        # ---------------- conv2 + avgpool16 -----------------
                        for c in range(2):
                            nc.vector.memset(pacc[img][c][:], 0.0)
                        for g in range(ngrp):
                            for co in range(2):
                                ps = [ps8.tile([128, NFREE], F32,
                                               name="psb", tag="psa")
                                      for _ in range(gs)]
                                for t in range(9):
                                    if mm_mode == "swi":
                                        wap = w2[:, t, co].rearrange(
                                            "q (a b) -> q a b", b=128)
                                    else:
                                        wap = w2[:, t, :,
                                                 co * 128:(co + 1) * 128]
                                    sh = (t // 3) * WP + t % 3
                                    for s in range(gs):
                                        off = RS * (g * gs + s) * WP + sh
                                        nc.tensor.matmul(
                                            ps[s][:], wap,
                                            ci[:, :, off:off + NFREE],
                                            start=(t == 0), stop=(t == 8),
                                            perf_mode=MM)
                                if corr and g == 0 and co == 0:
                                    # conv2 bias correction, emitted after
                                    # group-0 matmuls so the PE keeps
                                    # streaming while ACT/DVE settle hsum:
                                    # b2eff = b2 + selS @ mean(h1)
                                    for c in range(2):
                                        nc.vector.reduce_sum(
                                            hsum[img][c][:], hs28[c][:],
                                            axis=mybir.AxisListType.X)
                                    for c in range(2):
                                        pcc = ps8.tile([128, NFREE], F32,
                                                       name="pcc", tag="psa")
                                        nc.tensor.matmul(pcc[:, 0:1],
                                                         selS[:, 0, c, :],
                                                         hsum[img][0][:],
                                                         start=True,
                                                         stop=False)
                                        nc.tensor.matmul(pcc[:, 0:1],
                                                         selS[:, 1, c, :],
                                                         hsum[img][1][:],
                                                         start=False,
                                                         stop=True)
                                        nc.vector.tensor_add(
                                            b2eff[img][:, c:c + 1],
                                            pcc[:, 0:1], b2[:, c:c + 1])
                                for s in range(gs):
                                    sg = g * gs + s
                                    et = ob.tile([128, RS, 112], F32,
                                                 tag="et")
                                    nc.scalar.activation(
                                        et[:],
                                        ps[s][:].rearrange(
                                            "p (a b) -> p a b",
                                            b=WP)[:, :, 0:112],
                                        mybir.ActivationFunctionType.Relu,
                                        bias=(b2eff[img][:, co:co + 1]
                                              if corr else
                                              b2[:, co:co + 1]))
                                    # pool: XY-reduce over [4 rows x 16 cols]
                                    rp = ob.tile([128, 7], F32, tag="rp")
                                    nc.vector.reduce_sum(
                                        rp[:],
                                        et[:].rearrange(
                                            "p r (g c) -> p g r c", c=16),
                                        axis=mybir.AxisListType.XY)
                                    blk = sg // 4
                                    nc.vector.tensor_add(
                                        pacc[img][co][:, blk * 7:
                                                      (blk + 1) * 7],
                                        pacc[img][co][:, blk * 7:
                                                      (blk + 1) * 7],
                                        rp[:])

                # ---------------- phase C: head ----------------
                with (
                    tc.tile_pool(name="hc", bufs=cbufs) as hc,
                    tc.tile_pool(name="att", bufs=1) as attp,
                    tc.tile_pool(name="psc", bufs=1, space="PSUM") as psc,
                    tc.tile_pool(name="psco", bufs=2, space="PSUM") as psco,
                ):
                    # stacked per-(img) attention params [64, 1]
                    mxs = hc.tile([64, 1], F32, tag="mxs")
                    mys = hc.tile([64, 1], F32, tag="mys")
                    tts = hc.tile([64, 1], F32, tag="tts")
                    rhs_ = hc.tile([64, 1], F32, tag="rhs")

                    if "c" in phases:
                        # conv3 (7x7, 256->128), both images batched on the
                        # free dim: [128, 2(img), 63]
                        p3in = []
                        for ci in range(2):
                            pi = hc.tile([128, 2, 83], F32, tag=f"p3in{ci}")
                            nc.vector.memset(pi[:], 0.0)
                            for img in range(IMG):
                                nc.vector.tensor_copy(
                                    pi[:, img, 10:73].rearrange(
                                        "p (a b) -> p a b", b=9)[:, :, 0:7],
                                    pacc[img][ci][:].rearrange(
                                        "p (a b) -> p a b", b=7))
                            p3in.append(pi)
                        ps3 = psc.tile([128, 2, 63], F32, tag="ps3")
                        k = 0
                        for ci in range(2):
                            for t in range(9):
                                sh = (t // 3) * 9 + t % 3
                                nc.tensor.matmul(
                                    ps3[:], w3[:, t, ci, :],
                                    p3in[ci][:, :, sh:sh + 63],
                                    start=(k == 0), stop=(k == 17))
                                k += 1
                        p4in = hc.tile([128, 2, 83], F32, tag="p4in")
                        nc.vector.memset(p4in[:], 0.0)
                        nc.scalar.activation(
                            p4in[:, :, 10:73].rearrange(
                                "p i (a b) -> p i a b", b=9)[:, :, :, 0:7],
                            ps3[:].rearrange("p i (a b) -> p i a b",
                                             b=9)[:, :, :, 0:7],
                            mybir.ActivationFunctionType.Relu, bias=b3[:, 0:1])
                        # conv4 (7x7, 128->128)
                        ps4 = psc.tile([128, 2, 63], F32, tag="ps4")
                        for t in range(9):
                            sh = (t // 3) * 9 + t % 3
                            nc.tensor.matmul(
                                ps4[:], w4[:, t, :],
                                p4in[:, :, sh:sh + 63],
                                start=(t == 0), stop=(t == 8))
                        c4t = hc.tile([128, 2, 49], F32, tag="c4t")
                        nc.scalar.activation(
                            c4t[:].rearrange("p i (a b) -> p i a b", b=7),
                            ps4[:].rearrange("p i (a b) -> p i a b",
                                             b=9)[:, :, :, 0:7],
                            mybir.ActivationFunctionType.Relu, bias=b4[:, 0:1])
                        # avgpool 3x3 stride 2 (sum; /9 folded into w5)
                        c4v = c4t[:].rearrange("p i (y x) -> p i y x", x=7)
                        a1 = hc.tile([128, 2, 7, 3], F32, tag="a1")
                        nc.vector.tensor_add(a1[:], c4v[:, :, :, 0:5:2],
                                             c4v[:, :, :, 1:6:2])
                        nc.vector.tensor_add(a1[:], a1[:], c4v[:, :, :, 2:7:2])
                        a2 = hc.tile([128, 2, 9], F32, tag="a2")
                        a2v = a2[:].rearrange("p i (y x) -> p i y x", x=3)
                        nc.vector.tensor_add(a2v, a1[:, :, 0:5:2, :],
                                             a1[:, :, 1:6:2, :])
                        nc.vector.tensor_add(a2v, a2v, a1[:, :, 2:7:2, :])
                        # conv5 (3x3, 128->64)
                        p5in = hc.tile([128, 2, 27], F32, tag="p5in")
                        nc.vector.memset(p5in[:], 0.0)
                        nc.vector.tensor_copy(
                            p5in[:, :, 6:21].rearrange(
                                "p i (a b) -> p i a b", b=5)[:, :, :, 0:3],
                            a2[:].rearrange("p i (a b) -> p i a b", b=3))
                        ps5 = psc.tile([64, 2, 15], F32, tag="ps5")
                        for t in range(9):
                            sh = (t // 3) * 5 + t % 3
                            nc.tensor.matmul(
                                ps5[:], w5[:, t, :],
                                p5in[:, :, sh:sh + 15],
                                start=(t == 0), stop=(t == 8))
                        h5 = hc.tile([64, 2, 9], F32, tag="h5")
                        nc.scalar.activation(
                            h5[:].rearrange("p i (a b) -> p i a b", b=3),
                            ps5[:].rearrange("p i (a b) -> p i a b",
                                             b=5)[:, :, :, 0:3],
                            mybir.ActivationFunctionType.Relu, bias=b5[:, 0:1])
                        # fc 576->128 as 9 accumulating matmuls (K=64)
                        psf = psc.tile([128, 2], F32, tag="psf")
                        for t in range(9):
                            nc.tensor.matmul(psf[:], wfc[:, t, :],
                                             h5[:, :, t],
                                             start=(t == 0), stop=(t == 8))
                        sig = hc.tile([128, 2], F32, tag="sig")
                        nc.scalar.activation(sig[:], psf[:],
                                             mybir.ActivationFunctionType.
                                             Sigmoid)
                        # params: one selector matmul -> [mx | my | t | rho']
                        psl = psc.tile([128, 2], F32, tag="psl")
                        nc.tensor.matmul(psl[:], selp[:], sig[:],
                                         start=True, stop=True)
                        for img in range(IMG):
                            o = 32 * img
                            nc.vector.tensor_copy(mxs[o:o + 32],
                                                  psl[0:32, img:img + 1])
                            nc.vector.tensor_copy(mys[o:o + 32],
                                                  psl[32:64, img:img + 1])
                            nc.vector.tensor_copy(tts[o:o + 32],
                                                  psl[64:96, img:img + 1])
                            nc.vector.tensor_copy(rhs_[o:o + 32],
                                                  psl[96:128, img:img + 1])

                    # ---- batched attention for both images on 64 partitions
                    if "c" in phases:
                        r64 = hc.tile([64, 1], F32, tag="r64")
                        nc.scalar.activation(r64[:], tts[:],
                                             mybir.ActivationFunctionType.Exp,
                                             bias=cst[:, 0:1])
                        rho = hc.tile([64, 1], F32, tag="rho")
                        nc.vector.tensor_scalar(rho[:], rhs_[:], -0.8, None,
                                                mybir.AluOpType.add)
                        rr = hc.tile([64, 1], F32, tag="rr")
                        nc.vector.tensor_mul(rr[:], rho[:], rho[:])
                        om = hc.tile([64, 1], F32, tag="om")
                        nc.vector.tensor_scalar(om[:], rr[:], -1.0, 1.0,
                                                mybir.AluOpType.mult,
                                                mybir.AluOpType.add)
                        iom = hc.tile([64, 1], F32, tag="iom")
                        nc.vector.reciprocal(iom[:], om[:])
                        den = hc.tile([64, 1], F32, tag="den")
                        nc.vector.tensor_scalar(den[:], iom[:],
                                                -0.5 / (SIG2 * SIG2), None,
                                                mybir.AluOpType.mult)
                        ai = hc.tile([64, 1], F32, tag="ai")
                        nc.vector.tensor_mul(ai[:], den[:], r64[:])
                        ir = hc.tile([64, 1], F32, tag="ir")
                        nc.vector.reciprocal(ir[:], r64[:])
                        bj = hc.tile([64, 1], F32, tag="bj")
                        nc.vector.tensor_mul(bj[:], den[:], ir[:])
                        cc = hc.tile([64, 1], F32, tag="cc")
                        nc.vector.scalar_tensor_tensor(
                            cc[:], den[:], -2.0, rho[:],
                            mybir.AluOpType.mult, mybir.AluOpType.mult)
                        dx = hc.tile([64, 56], F32, tag="dx")
                        nc.vector.tensor_scalar(dx[:], negio[:], mxs[:, 0:1],
                                                None, mybir.AluOpType.add)
                        dy = hc.tile([64, 56], F32, tag="dy")
                        nc.vector.tensor_scalar(dy[:], negio[:], mys[:, 0:1],
                                                None, mybir.AluOpType.add)
                        u = hc.tile([64, 56], F32, tag="u")
                        nc.vector.scalar_tensor_tensor(
                            u[:], dx[:], ai[:, 0:1], dx[:],
                            mybir.AluOpType.mult, mybir.AluOpType.mult)
                        v = hc.tile([64, 56], F32, tag="v")
                        nc.vector.scalar_tensor_tensor(
                            v[:], dy[:], bj[:, 0:1], dy[:],
                            mybir.AluOpType.mult, mybir.AluOpType.mult)
                        # build the quadratic form in row blocks so DVE
                        # pipeline drains overlap across independent blocks
                        lt = attp.tile([64, 56, 56], F32, tag="lt")
                        rb = 56 // nblk
                        blks = [(i * rb, (i + 1) * rb) for i in range(nblk)]
                        for r0, r1 in blks:
                            nc.vector.scalar_tensor_tensor(
                                lt[:, r0:r1],
                                dx[:, r0:r1].unsqueeze(2).broadcast_to(
                                    [64, rb, 56]),
                                cc[:, 0:1],
                                dy[:].unsqueeze(1).broadcast_to([64, rb, 56]),
                                mybir.AluOpType.mult, mybir.AluOpType.mult)
                        for r0, r1 in blks:
                            nc.gpsimd.tensor_add(
                                lt[:, r0:r1], lt[:, r0:r1],
                                u[:, r0:r1].unsqueeze(2).broadcast_to(
                                    [64, rb, 56]))
                        for r0, r1 in blks:
                            nc.vector.tensor_add(
                                lt[:, r0:r1], lt[:, r0:r1],
                                v[:].unsqueeze(1).broadcast_to([64, rb, 56]))
                        att = attp.tile([64, 56 * 56], F32, tag="att")
                        asums = hc.tile([64, nblk], F32, tag="asums")
                        for i, (r0, r1) in enumerate(blks):
                            nc.scalar.activation(
                                att[:, r0 * 56:r1 * 56],
                                lt[:, r0:r1].rearrange("p a b -> p (a b)"),
                                mybir.ActivationFunctionType.Exp,
                                accum_out=asums[:, i:i + 1])
                        asum = hc.tile([64, 1], F32, tag="asum")
                        nc.vector.reduce_sum(asum[:], asums[:],
                                             axis=mybir.AxisListType.X)
                        inv = hc.tile([64, 1], F32, tag="inv")
                        nc.vector.reciprocal(inv[:], asum[:])
                        # fold row normalization into the mixture selector
                        sg2 = hc.tile([64, 16], F32, tag="sg2")
                        nc.vector.tensor_scalar(sg2[:], selg[:], inv[:, 0:1],
                                                None, mybir.AluOpType.mult)
                        obuf = attp.tile([16, 56 * 56], F32, tag="obuf")
                        for ch in range(7):
                            pso = psco.tile([16, 448], F32, tag="pso")
                            nc.tensor.matmul(pso[:], sg2[:],
                                             att[:, ch * 448:(ch + 1) * 448],
                                             start=True, stop=True)
                            nc.vector.tensor_copy(
                                obuf[:, ch * 448:(ch + 1) * 448], pso[:])
                        nc.gpsimd.dma_start(
                            out.rearrange("i o a b -> (i o) (a b)"), obuf[:])

            if r_loop:
                with tc.For_i(0, r_loop, 1):
                    emit_body()
            else:
                emit_body()
    nc.compile()
    return nc


def prep_inputs(inputs, conv_dtype="fp8", mm_mode="swi"):
    """Host prep: fold BN/pool scales, quantize, build layouts, shard batch."""
    import ml_dtypes
    F8NP = ml_dtypes.float8_e4m3

    x = inputs["x"]
    eps_s = 1.0 / np.sqrt(np.float32(1.0 + 1e-5))

    def fold(w, g):
        s = (g * eps_s).astype(np.float32)
        return (w * s[:, None, None, None]).astype(np.float32)

    w1 = fold(inputs["w1"], inputs["g1"])            # [256,512,3,3]
    w2 = fold(inputs["w2"], inputs["g2"])            # [256,256,3,3]
    w3 = fold(inputs["w3"], inputs["g3"]) / 256.0    # avgpool16 norm
    w4 = fold(inputs["w4"], inputs["g4"])
    w5 = fold(inputs["w5"], inputs["g5"]) / 9.0      # avgpool3 norm
    wfc = np.asarray(inputs["w_fc"], np.float32)     # [128, 576]
    mw = np.asarray(inputs["mix_w"], np.float32).reshape(OUT, GMM)
    mw = np.exp(mw - mw.max(1, keepdims=True))
    mw = mw / mw.sum(1, keepdims=True)               # softmax over gmm

    w2q = w2.astype(F8NP).astype(np.float32)
    # conv2 fp8 mean-correction: corr[co] = S @ mean(h1), S = sum_taps dw2
    S = (w2 - w2q).sum(axis=(2, 3)) / float(H * W)   # [co, ci]
    selS = np.ascontiguousarray(
        S.reshape(2, 128, 2, 128).transpose(3, 2, 0, 1))  # [cip,cic,coc,cop]

    # conv weights -> [128(p=cin%128), 9(tap), ncin, cout]
    def wt_layout(w, ncin):
        co = w.shape[0]
        r = w.transpose(1, 2, 3, 0).reshape(ncin, 128, 9, co)
        return np.ascontiguousarray(r.transpose(1, 2, 0, 3))

    def swi_layout(wt, npair):
        # wt: [128, 9, ncin, co]; out: [128, 9, npair(, 2coc), 256] where the
        # last dim holds (A[127-j], B[127-j]) interleaved pairs per co chunk
        ncin, co = wt.shape[2], wt.shape[3]
        ncoc = co // 128
        w = wt.reshape(128, 9, npair, 2, ncoc, 128)       # [.., pair, b, coc, cow]
        w = w[..., ::-1]                                   # reverse cow
        w = w.transpose(0, 1, 2, 4, 5, 3)                  # [.., pair, coc, cow, b]
        out = w.reshape(128, 9, npair, ncoc, 256)
        if npair == 1:
            out = out[:, :, 0]
        return np.ascontiguousarray(out)

    w1f = wt_layout(w1.astype(F8NP).astype(np.float32), 4)
    w2f = wt_layout(w2q, 2)
    if mm_mode == "swi":
        w1t = swi_layout(w1f, 2).astype(F8NP)
        w2t = swi_layout(w2f, 1).astype(F8NP)
    else:
        w1t = w1f.astype(F8NP)
        w2t = w2f.astype(F8NP)
    w3t = wt_layout(w3, 2)
    w4t = wt_layout(w4, 1)[:, :, 0, :]
    w5t = wt_layout(w5, 1)[:, :, 0, :]
    wfct = np.ascontiguousarray(wfc.reshape(128, 64, 9).transpose(1, 2, 0))

    def bias_chunks(b, nchunk):
        return np.ascontiguousarray(
            np.asarray(b, np.float32).reshape(nchunk, 128).T)

    b1h = bias_chunks(inputs["b1"], 2)
    b2h = bias_chunks(inputs["b2"], 2)
    b3h = np.asarray(inputs["b3"], np.float32).reshape(128, 1)
    b4h = np.asarray(inputs["b4"], np.float32).reshape(128, 1)
    b5h = np.asarray(inputs["b5"], np.float32).reshape(64, 1)

    selp = np.zeros((128, 128), np.float32)
    for m in range(32):
        selp[4 * m + 0, m] = float(H2 - 1)
        selp[4 * m + 1, m + 32] = float(W2 - 1)
        selp[4 * m + 2, m + 64] = 2.0 * LOGR
        selp[4 * m + 3, m + 96] = 1.6
    selg = np.zeros((64, 16), np.float32)
    for i in range(IMG):
        for o in range(OUT):
            for g in range(GMM):
                selg[32 * i + o * GMM + g, 8 * i + o] = mw[o, g]
    negio = np.broadcast_to(-np.arange(56, dtype=np.float32), (64, 56)).copy()
    cst = np.full((64, 1), -LOGR, np.float32)

    xp = np.zeros((B, 4, 128, HP, WP), F8NP)
    xp[:, :, :, 1:113, 1:113] = np.asarray(x, np.float32).reshape(
        B, 4, 128, H, W).astype(F8NP)

    common = {
        "w1t": w1t, "w2t": w2t,
        "w3t": w3t, "w4t": w4t, "w5t": w5t, "wfct": wfct,
        "b1d": b1h, "b2d": b2h, "b3d": b3h, "b4d": b4h, "b5d": b5h,
        "selpd": selp, "selgd": selg, "negiod": negio, "cstd": cst,
        "selSd": selS,
    }
    in_maps = []
    for c in range(NCORE):
        m = dict(common)
        m["x"] = np.ascontiguousarray(xp[c * IMG:(c + 1) * IMG])
        in_maps.append(m)
    return in_maps


_CACHE = {}


def kernel(**inputs):
    inputs = {k: np.asarray(v) for k, v in inputs.items()}
    if "nc" not in _CACHE:
        _CACHE["nc"] = build_nc()
    nc = _CACHE["nc"]
    in_maps = prep_inputs(inputs)
    res = run_bass_kernel_spmd(nc, in_maps, core_ids=list(range(NCORE)))
    out = np.concatenate([res.results[c]["out"] for c in range(NCORE)], axis=0)
    return np.ascontiguousarray(out.astype(np.float32))


# revision 27
# speedup vs baseline: 1.0013x; 1.0013x over previous
"""Trainium2 Bass kernel for nn_BivariateNormalAttention.

Self-contained: takes FULL inputs (B=16), shards batch across 8 NeuronCores
(2 images/core), runs a Bass/Tile kernel per core, gathers [16,8,56,56].

Pipeline per image:
  conv3x3(512->256)+BN+ReLU -> conv3x3(256->256)+BN+ReLU -> avgpool16 (7x7)
  -> conv3x3(256->128)+BN+ReLU -> conv3x3(128->128)+BN+ReLU -> avgpool3s2 (3x3)
  -> conv3x3(128->64)+BN+ReLU -> fc(576->128) -> bivariate-normal attention maps.

Convs 1-2 (99.9% of FLOPs) run as 9-tap shifted matmuls in fp8-e4m3 with
perf_mode=DoubleRow (two 128-deep K blocks per instruction at 0.5 cyc/row).
Weight loads amortize over groups of 4 row-strips. The fp8 weight-quantization
bias (which couples to the nonzero mean of post-ReLU activations) is removed
by a per-image bias correction: corr = (sum_taps dw2) @ mean(h1), computed on
device from a running sum of conv1 activations. Convs 3-5 / fc / attention run
in fp32; the attention tail batches both images on 64 partitions.
"""
import sys
import numpy as np

for _p in ("/opt/trn_rl_repo", "/root/.axon_site/_ro/trn_rl_repo"):
    if _p not in sys.path:
        sys.path.append(_p)

import concourse.bacc as bacc
import concourse.mybir as mybir
import concourse.tile as tile
from concourse.bass_utils import run_bass_kernel_spmd

F32 = mybir.dt.float32
F8 = mybir.dt.float8e4
DR = mybir.MatmulPerfMode.DoubleRow
DRS = mybir.MatmulPerfMode.DoubleRowSwInterleave

B, C, H, W = 16, 512, 112, 112
OUT, GMM = 8, 4
NCORE = 8
IMG = B // NCORE                 # 2 images per core
HP, WP = H + 2, W + 2            # 114
FLAT = HP * WP                   # 12996
RS = 4                           # conv strip rows
NSTRIP = H // RS                 # 28
GS = 4                           # strips per weight-load group
NGRP = NSTRIP // GS              # 7
NFREE = RS * WP                  # 456
XL = 688                         # per-ci lane length (6*WP+2=686 padded to 16)
H2 = W2 = H // 2                 # 56
SIG2 = float(H) / 2.0            # sigma = 56
LOGR = float(np.log(3.0))


def build_nc(conv_dtype="fp8", r_loop=None, ldma="whole4", xbufs=8, pbufs=8,
             cbufs=2, phases="abc", mm_mode="swi", gs=4, corr=True,
             sorder="ki", skel=None, nblk=4, obufs=12):
    """Bass program for one core processing IMG images."""
    nc = bacc.Bacc("TRN2", target_bir_lowering=False, debug=False)

    MM = DRS if mm_mode == "swi" else DR
    x = nc.dram_tensor("x", [IMG, 4, 128, HP, WP], F8, kind="ExternalInput")
    if mm_mode == "swi":
        w1t = nc.dram_tensor("w1t", [128, 9, 2, 2, 256], F8,
                             kind="ExternalInput")
        w2t = nc.dram_tensor("w2t", [128, 9, 2, 256], F8,
                             kind="ExternalInput")
    else:
        w1t = nc.dram_tensor("w1t", [128, 9, 4, 256], F8,
                             kind="ExternalInput")
        w2t = nc.dram_tensor("w2t", [128, 9, 2, 256], F8,
                             kind="ExternalInput")
    w3t = nc.dram_tensor("w3t", [128, 9, 2, 128], F32, kind="ExternalInput")
    w4t = nc.dram_tensor("w4t", [128, 9, 128], F32, kind="ExternalInput")
    w5t = nc.dram_tensor("w5t", [128, 9, 64], F32, kind="ExternalInput")
    wfct = nc.dram_tensor("wfct", [64, 9, 128], F32, kind="ExternalInput")
    b1d = nc.dram_tensor("b1d", [128, 2], F32, kind="ExternalInput")
    b2d = nc.dram_tensor("b2d", [128, 2], F32, kind="ExternalInput")
    b3d = nc.dram_tensor("b3d", [128, 1], F32, kind="ExternalInput")
    b4d = nc.dram_tensor("b4d", [128, 1], F32, kind="ExternalInput")
    b5d = nc.dram_tensor("b5d", [64, 1], F32, kind="ExternalInput")
    selpd = nc.dram_tensor("selpd", [128, 128], F32, kind="ExternalInput")
    selgd = nc.dram_tensor("selgd", [64, 16], F32, kind="ExternalInput")
    negiod = nc.dram_tensor("negiod", [64, 56], F32, kind="ExternalInput")
    cstd = nc.dram_tensor("cstd", [64, 1], F32, kind="ExternalInput")  # -ln3
    selSd = nc.dram_tensor("selSd", [128, 2, 2, 128], F32,
                           kind="ExternalInput")  # conv2 fp8 mean correction

    out = nc.dram_tensor("out", [IMG, OUT, H2, W2], F32, kind="ExternalOutput")

    ldma_eng = {"gpsimd": nc.gpsimd,
                "sync": nc.sync}.get(ldma, nc.sync)

    with tile.TileContext(nc) as tc:
        with tc.tile_pool(name="persist", bufs=1) as pp:
            b1 = pp.tile([128, 2], F32)
            b2 = pp.tile([128, 2], F32)
            selS = pp.tile([128, 2, 2, 128], F32)
            if mm_mode == "swi":
                w1 = pp.tile([128, 9, 2, 2, 256], F8, name="w1")
            else:
                w1 = pp.tile([128, 9, 4, 256], F8, name="w1")
            w2 = pp.tile([128, 9, 2, 256], F8, name="w2")
            w3 = pp.tile([128, 9, 2, 128], F32, name="w3")
            w4 = pp.tile([128, 9, 128], F32, name="w4")
            w5 = pp.tile([128, 9, 64], F32, name="w5")
            wfc = pp.tile([64, 9, 128], F32, name="wfc")
            b3 = pp.tile([128, 1], F32, name="b3")
            b4 = pp.tile([128, 1], F32, name="b4")
            b5 = pp.tile([64, 1], F32, name="b5")
            selp = pp.tile([128, 128], F32, name="selp")
            selg = pp.tile([64, 16], F32, name="selg")
            negio = pp.tile([64, 56], F32, name="negio")
            cst = pp.tile([64, 1], F32, name="cst")
            # constants loaded ONCE per launch (outside the timing loop)
            nc.gpsimd.dma_start(w1[:], w1t[:])
            nc.gpsimd.dma_start(w2[:], w2t[:])
            nc.gpsimd.dma_start(b1[:], b1d[:])
            nc.gpsimd.dma_start(b2[:], b2d[:])
            nc.gpsimd.dma_start(selS[:], selSd[:])
            for tdst, tsrc in ((w3, w3t), (w4, w4t), (w5, w5t),
                               (wfc, wfct), (b3, b3d), (b4, b4d),
                               (b5, b5d), (selp, selpd), (selg, selgd),
                               (negio, negiod), (cst, cstd)):
                nc.gpsimd.dma_start(tdst[:], tsrc[:])
            # per-(img, chunk) sums of conv1 activations / pool accums
            hsum = [[pp.tile([128, 1], F32, name=f"hs{i}_{c}",
                             tag=f"hs{i}_{c}")
                     for c in range(2)] for i in range(IMG)]
            b2eff = [pp.tile([128, 2], F32, name=f"b2e{i}",
                             tag=f"b2e{i}")
                     for i in range(IMG)]
            pacc = [[pp.tile([128, 49], F32, name=f"pacc{i}_{c}",
                             tag=f"pacc{i}_{c}")
                     for c in range(2)] for i in range(IMG)]

            def emit_body():

                # ---- phases A+B per image, x and conv1 output resident in
                # SBUF (c1 intermediate never round-trips through DRAM)
                XIL = 13008  # FLAT=12996 padded to a 16B multiple
                with (
                    tc.tile_pool(name="xpi", bufs=2) as xpi,
                    tc.tile_pool(name="cpi", bufs=2) as cpi,
                    tc.tile_pool(name="oa", bufs=4) as oa,
                    tc.tile_pool(name="ob", bufs=6) as ob,
                    tc.tile_pool(name="ps8", bufs=pbufs, space="PSUM") as ps8,
                ):
                    ngrp = NSTRIP // gs
                    for img in range(IMG):
                        xi = xpi.tile([128, 4, XIL], F8, tag="xi")
                        ci = cpi.tile([128, 2, XIL], F8, tag="ci")
                        xflat = x[img].rearrange("c p a b -> p c (a b)")
                        civ = ci[:, :, 0:FLAT].rearrange(
                            "p c (a b) -> p c a b", b=WP)
                        nc.vector.memset(xi[:, :, FLAT:XIL], 0.0)
                        nc.vector.memset(ci[:, :, FLAT:XIL], 0.0)
                        nc.vector.memset(civ[:, :, 0, :], 0.0)
                        nc.vector.memset(civ[:, :, HP - 1, :], 0.0)
                        nc.vector.memset(civ[:, :, :, 0], 0.0)
                        nc.vector.memset(civ[:, :, :, WP - 1], 0.0)
                        hs28 = [oa.tile([128, NSTRIP], F32, name=f"hs28_{c}",
                                        tag=f"hs28_{c}") for c in range(2)]

                        # ---------------- conv1 (fp8 DoubleRow) -------------
                        # group-size ramp: tiny first groups let the PE start
                        # ~0.5us into the (barrier-gated) x load instead of
                        # waiting for a quarter-image chunk
                        if ldma == "ramp":
                            plan = [1, 1, 2] + [gs] * ((NSTRIP - 4) // gs)
                        else:
                            plan = [gs] * (NSTRIP // gs)
                        starts = [sum(plan[:j]) for j in range(len(plan))]
                        if "a" in phases:
                            if ldma == "ramp":
                                # chunk cuts matched to each group's rows
                                cuts = [0]
                                for j, n in enumerate(plan):
                                    e = starts[j] + n
                                    cuts.append(min(FLAT,
                                                    (4 * e + 2) * WP))
                                for i in range(len(plan)):
                                    if cuts[i + 1] > cuts[i]:
                                        nc.sync.dma_start(
                                            xi[:, :, cuts[i]:cuts[i + 1]],
                                            xflat[:, :, cuts[i]:cuts[i + 1]])
                            elif ldma.startswith("whole"):
                                spec = ldma[5:]
                                par = spec.endswith("q")
                                nchunk = int(spec.rstrip("q") or 1)
                                qs = ([nc.sync, nc.gpsimd, nc.scalar]
                                      if par else [nc.sync])
                                cuts = [0] + [
                                    ((FLAT * (i + 1) // nchunk) // 16) * 16
                                    for i in range(nchunk - 1)] + [FLAT]
                                for i in range(nchunk):
                                    qs[i % len(qs)].dma_start(
                                        xi[:, :, cuts[i]:cuts[i + 1]],
                                        xflat[:, :, cuts[i]:cuts[i + 1]])
                            for gj, gn in enumerate(plan):
                                g0 = starts[gj]
                                if not (ldma.startswith("whole")
                                        or ldma == "ramp"):
                                    if gj == 0:
                                        lo, hi = 0, (4 * gn + 2) * WP
                                    else:
                                        lo = (4 * g0 + 2) * WP
                                        hi = min(FLAT,
                                                 (4 * (g0 + gn) + 2) * WP)
                                    ldma_eng.dma_start(xi[:, :, lo:hi],
                                                       xflat[:, :, lo:hi])
                                for co in range(2):
                                    ps = [ps8.tile([128, NFREE], F32,
                                                   name="psa", tag="psa")
                                          for _ in range(gn)]
                                    for k in range(18):
                                        p, t = divmod(k, 9)
                                        if mm_mode == "swi":
                                            wap = w1[:, t, p, co].rearrange(
                                                "q (a b) -> q a b", b=128)
                                        else:
                                            wap = w1[:, t, 2 * p:2 * p + 2,
                                                     co * 128:(co + 1) * 128]
                                        sh = (t // 3) * WP + t % 3
                                        for s in range(gn):
                                            off = RS * (g0 + s) * WP + sh
                                            nc.tensor.matmul(
                                                ps[s][:], wap,
                                                xi[:, 2 * p:2 * p + 2,
                                                   off:off + NFREE],
                                                start=(k == 0),
                                                stop=(k == 17),
                                                perf_mode=MM)
                                    for s in range(gn):
                                        sg = g0 + s
                                        base = (1 + RS * sg) * WP
                                        if skel == "mmonly":
                                            ob_ = oa.tile([128, NFREE], F32,
                                                          tag="ob_")
                                            nc.vector.tensor_copy(ob_[:],
                                                                  ps[s][:])
                                            continue
                                        nc.scalar.activation(
                                            ci[:, co, base:base + NFREE]
                                            .rearrange("p (a b) -> p a b",
                                                       b=WP)[:, :, 1:113],
                                            ps[s][:].rearrange(
                                                "p (a b) -> p a b",
                                                b=WP)[:, :, 0:112],
                                            mybir.ActivationFunctionType.Relu,
                                            bias=b1[:, co:co + 1],
                                            accum_out=(hs28[co][:, sg:sg + 1]
                                                       if corr else None))

                        if "b" not in phases:
                            continue
                        # ---------------- conv2 + avgpool16 -----------------
                        for c in range(2):
                            nc.vector.memset(pacc[img][c][:], 0.0)
                        for g in range(ngrp):
                            for co in range(2):
                                ps = [ps8.tile([128, NFREE], F32,
                                               name="psb", tag="psa")
                                      for _ in range(gs)]
                                for t in range(9):
                                    if mm_mode == "swi":
                                        wap = w2[:, t, co].rearrange(
                                            "q (a b) -> q a b", b=128)
                                    else:
                                        wap = w2[:, t, :,
                                                 co * 128:(co + 1) * 128]
                                    sh = (t // 3) * WP + t % 3
                                    for s in range(gs):
                                        off = RS * (g * gs + s) * WP + sh
                                        nc.tensor.matmul(
                                            ps[s][:], wap,
                                            ci[:, :, off:off + NFREE],
                                            start=(t == 0), stop=(t == 8),
                                            perf_mode=MM)
                                if corr and g == 0 and co == 0:
                                    # conv2 bias correction, emitted after
                                    # group-0 matmuls so the PE keeps
                                    # streaming while ACT/DVE settle hsum:
                                    # b2eff = b2 + selS @ mean(h1)
                                    for c in range(2):
                                        nc.vector.reduce_sum(
                                            hsum[img][c][:], hs28[c][:],
                                            axis=mybir.AxisListType.X)
                                    for c in range(2):
                                        pcc = ps8.tile([128, NFREE], F32,
                                                       name="pcc", tag="psa")
                                        nc.tensor.matmul(pcc[:, 0:1],
                                                         selS[:, 0, c, :],
                                                         hsum[img][0][:],
                                                         start=True,
                                                         stop=False)
                                        nc.tensor.matmul(pcc[:, 0:1],
                                                         selS[:, 1, c, :],
                                                         hsum[img][1][:],
                                                         start=False,
                                                         stop=True)
                                        nc.vector.tensor_add(
                                            b2eff[img][:, c:c + 1],
                                            pcc[:, 0:1], b2[:, c:c + 1])
                                for s in range(gs):
                                    sg = g * gs + s
                                    et = ob.tile([128, RS, 112], F32,
                                                 tag="et")
                                    nc.scalar.activation(
                                        et[:],
                                        ps[s][:].rearrange(
                                            "p (a b) -> p a b",
                                            b=WP)[:, :, 0:112],
                                        mybir.ActivationFunctionType.Relu,
                                        bias=(b2eff[img][:, co:co + 1]
                                              if corr else
                                              b2[:, co:co + 1]))
                                    # pool: XY-reduce over [4 rows x 16 cols]
                                    rp = ob.tile([128, 7], F32, tag="rp")
                                    nc.vector.reduce_sum(
                                        rp[:],
                                        et[:].rearrange(
                                            "p r (g c) -> p g r c", c=16),
                                        axis=mybir.AxisListType.XY)
                                    blk = sg // 4
                                    nc.vector.tensor_add(
                                        pacc[img][co][:, blk * 7:
                                                      (blk + 1) * 7],
                                        pacc[img][co][:, blk * 7:
                                                      (blk + 1) * 7],
                                        rp[:])

                # ---------------- phase C: head ----------------
                with (
                    tc.tile_pool(name="hc", bufs=cbufs) as hc,
                    tc.tile_pool(name="att", bufs=1) as attp,
                    tc.tile_pool(name="psc", bufs=1, space="PSUM") as psc,
                    tc.tile_pool(name="psco", bufs=2, space="PSUM") as psco,
                ):
                    # stacked per-(img) attention params [64, 1]
                    mxs = hc.tile([64, 1], F32, tag="mxs")
                    mys = hc.tile([64, 1], F32, tag="mys")
                    tts = hc.tile([64, 1], F32, tag="tts")
                    rhs_ = hc.tile([64, 1], F32, tag="rhs")

                    if "c" in phases:
                        # conv3 (7x7, 256->128), both images batched on the
                        # free dim: [128, 2(img), 63]
                        p3in = []
                        for ci in range(2):
                            pi = hc.tile([128, 2, 83], F32, tag=f"p3in{ci}")
                            nc.vector.memset(pi[:], 0.0)
                            for img in range(IMG):
                                nc.vector.tensor_copy(
                                    pi[:, img, 10:73].rearrange(
                                        "p (a b) -> p a b", b=9)[:, :, 0:7],
                                    pacc[img][ci][:].rearrange(
                                        "p (a b) -> p a b", b=7))
                            p3in.append(pi)
                        ps3 = psc.tile([128, 2, 63], F32, tag="ps3")
                        k = 0
                        for ci in range(2):
                            for t in range(9):
                                sh = (t // 3) * 9 + t % 3
                                nc.tensor.matmul(
                                    ps3[:], w3[:, t, ci, :],
                                    p3in[ci][:, :, sh:sh + 63],
                                    start=(k == 0), stop=(k == 17))
                                k += 1
                        p4in = hc.tile([128, 2, 83], F32, tag="p4in")
                        nc.vector.memset(p4in[:], 0.0)
                        nc.scalar.activation(
                            p4in[:, :, 10:73].rearrange(
                                "p i (a b) -> p i a b", b=9)[:, :, :, 0:7],
                            ps3[:].rearrange("p i (a b) -> p i a b",
                                             b=9)[:, :, :, 0:7],
                            mybir.ActivationFunctionType.Relu, bias=b3[:, 0:1])
                        # conv4 (7x7, 128->128)
                        ps4 = psc.tile([128, 2, 63], F32, tag="ps4")
                        for t in range(9):
                            sh = (t // 3) * 9 + t % 3
                            nc.tensor.matmul(
                                ps4[:], w4[:, t, :],
                                p4in[:, :, sh:sh + 63],
                                start=(t == 0), stop=(t == 8))
                        c4t = hc.tile([128, 2, 49], F32, tag="c4t")
                        nc.scalar.activation(
                            c4t[:].rearrange("p i (a b) -> p i a b", b=7),
                            ps4[:].rearrange("p i (a b) -> p i a b",
                                             b=9)[:, :, :, 0:7],
                            mybir.ActivationFunctionType.Relu, bias=b4[:, 0:1])
                        # avgpool 3x3 stride 2 (sum; /9 folded into w5)
                        c4v = c4t[:].rearrange("p i (y x) -> p i y x", x=7)
                        a1 = hc.tile([128, 2, 7, 3], F32, tag="a1")
                        nc.vector.tensor_add(a1[:], c4v[:, :, :, 0:5:2],
                                             c4v[:, :, :, 1:6:2])
                        nc.vector.tensor_add(a1[:], a1[:], c4v[:, :, :, 2:7:2])
                        a2 = hc.tile([128, 2, 9], F32, tag="a2")
                        a2v = a2[:].rearrange("p i (y x) -> p i y x", x=3)
                        nc.vector.tensor_add(a2v, a1[:, :, 0:5:2, :],
                                             a1[:, :, 1:6:2, :])
                        nc.vector.tensor_add(a2v, a2v, a1[:, :, 2:7:2, :])
                        # conv5 (3x3, 128->64)
                        p5in = hc.tile([128, 2, 27], F32, tag="p5in")
                        nc.vector.memset(p5in[:], 0.0)
                        nc.vector.tensor_copy(
                            p5in[:, :, 6:21].rearrange(
                                "p i (a b) -> p i a b", b=5)[:, :, :, 0:3],
                            a2[:].rearrange("p i (a b) -> p i a b", b=3))
                        ps5 = psc.tile([64, 2, 15], F32, tag="ps5")
                        for t in range(9):
                            sh = (t // 3) * 5 + t % 3
                            nc.tensor.matmul(
                                ps5[:], w5[:, t, :],
                                p5in[:, :, sh:sh + 15],
                                start=(t == 0), stop=(t == 8))
                        h5 = hc.tile([64, 2, 9], F32, tag="h5")
                        nc.scalar.activation(
                            h5[:].rearrange("p i (a b) -> p i a b", b=3),
                            ps5[:].rearrange("p i (a b) -> p i a b",
                                             b=5)[:, :, :, 0:3],
                            mybir.ActivationFunctionType.Relu, bias=b5[:, 0:1])
                        # fc 576->128 as 9 accumulating matmuls (K=64)
                        psf = psc.tile([128, 2], F32, tag="psf")
                        for t in range(9):
                            nc.tensor.matmul(psf[:], wfc[:, t, :],
                                             h5[:, :, t],
                                             start=(t == 0), stop=(t == 8))
                        sig = hc.tile([128, 2], F32, tag="sig")
                        nc.scalar.activation(sig[:], psf[:],
                                             mybir.ActivationFunctionType.
                                             Sigmoid)
                        # params: one selector matmul -> [mx | my | t | rho']
                        psl = psc.tile([128, 2], F32, tag="psl")
                        nc.tensor.matmul(psl[:], selp[:], sig[:],
                                         start=True, stop=True)
                        for img in range(IMG):
                            o = 32 * img
                            nc.vector.tensor_copy(mxs[o:o + 32],
                                                  psl[0:32, img:img + 1])
                            nc.vector.tensor_copy(mys[o:o + 32],
                                                  psl[32:64, img:img + 1])
                            nc.vector.tensor_copy(tts[o:o + 32],
                                                  psl[64:96, img:img + 1])
                            nc.vector.tensor_copy(rhs_[o:o + 32],
                                                  psl[96:128, img:img + 1])

                    # ---- batched attention for both images on 64 partitions
                    if "c" in phases:
                        r64 = hc.tile([64, 1], F32, tag="r64")
                        nc.scalar.activation(r64[:], tts[:],
                                             mybir.ActivationFunctionType.Exp,
                                             bias=cst[:, 0:1])
                        rho = hc.tile([64, 1], F32, tag="rho")
                        nc.vector.tensor_scalar(rho[:], rhs_[:], -0.8, None,
                                                mybir.AluOpType.add)
                        rr = hc.tile([64, 1], F32, tag="rr")
                        nc.vector.tensor_mul(rr[:], rho[:], rho[:])
                        om = hc.tile([64, 1], F32, tag="om")
                        nc.vector.tensor_scalar(om[:], rr[:], -1.0, 1.0,
                                                mybir.AluOpType.mult,
                                                mybir.AluOpType.add)
                        iom = hc.tile([64, 1], F32, tag="iom")
                        nc.vector.reciprocal(iom[:], om[:])
                        den = hc.tile([64, 1], F32, tag="den")
                        nc.vector.tensor_scalar(den[:], iom[:],
                                                -0.5 / (SIG2 * SIG2), None,
                                                mybir.AluOpType.mult)
                        ai = hc.tile([64, 1], F32, tag="ai")
                        nc.vector.tensor_mul(ai[:], den[:], r64[:])
                        ir = hc.tile([64, 1], F32, tag="ir")
                        nc.vector.reciprocal(ir[:], r64[:])
                        bj = hc.tile([64, 1], F32, tag="bj")
                        nc.vector.tensor_mul(bj[:], den[:], ir[:])
                        cc = hc.tile([64, 1], F32, tag="cc")
                        nc.vector.scalar_tensor_tensor(
                            cc[:], den[:], -2.0, rho[:],
                            mybir.AluOpType.mult, mybir.AluOpType.mult)
                        dx = hc.tile([64, 56], F32, tag="dx")
                        nc.vector.tensor_scalar(dx[:], negio[:], mxs[:, 0:1],
                                                None, mybir.AluOpType.add)
                        dy = hc.tile([64, 56], F32, tag="dy")
                        nc.vector.tensor_scalar(dy[:], negio[:], mys[:, 0:1],
                                                None, mybir.AluOpType.add)
                        u = hc.tile([64, 56], F32, tag="u")
                        nc.vector.scalar_tensor_tensor(
                            u[:], dx[:], ai[:, 0:1], dx[:],
                            mybir.AluOpType.mult, mybir.AluOpType.mult)
                        v = hc.tile([64, 56], F32, tag="v")
                        nc.vector.scalar_tensor_tensor(
                            v[:], dy[:], bj[:, 0:1], dy[:],
                            mybir.AluOpType.mult, mybir.AluOpType.mult)
                        # build the quadratic form in row blocks so DVE
                        # pipeline drains overlap across independent blocks
                        lt = attp.tile([64, 56, 56], F32, tag="lt")
                        rb = 56 // nblk
                        blks = [(i * rb, (i + 1) * rb) for i in range(nblk)]
                        for r0, r1 in blks:
                            nc.vector.scalar_tensor_tensor(
                                lt[:, r0:r1],
                                dx[:, r0:r1].unsqueeze(2).broadcast_to(
                                    [64, rb, 56]),
                                cc[:, 0:1],
                                dy[:].unsqueeze(1).broadcast_to([64, rb, 56]),
                                mybir.AluOpType.mult, mybir.AluOpType.mult)
                        for r0, r1 in blks:
                            nc.gpsimd.tensor_add(
                                lt[:, r0:r1], lt[:, r0:r1],
                                u[:, r0:r1].unsqueeze(2).broadcast_to(
                                    [64, rb, 56]))
                        for r0, r1 in blks:
                            nc.vector.tensor_add(
                                lt[:, r0:r1], lt[:, r0:r1],
                                v[:].unsqueeze(1).broadcast_to([64, rb, 56]))
                        att = attp.tile([64, 56 * 56], F32, tag="att")
                        asums = hc.tile([64, nblk], F32, tag="asums")
                        for i, (r0, r1) in enumerate(blks):
                            nc.scalar.activation(
                                att[:, r0 * 56:r1 * 56],
                                lt[:, r0:r1].rearrange("p a b -> p (a b)"),
                                mybir.ActivationFunctionType.Exp,
                                accum_out=asums[:, i:i + 1])
                        asum = hc.tile([64, 1], F32, tag="asum")
                        nc.vector.reduce_sum(asum[:], asums[:],
                                             axis=mybir.AxisListType.X)
                        inv = hc.tile([64, 1], F32, tag="inv")
                        nc.vector.reciprocal(inv[:], asum[:])
                        # fold row normalization into the mixture selector
                        sg2 = hc.tile([64, 16], F32, tag="sg2")
                        nc.vector.tensor_scalar(sg2[:], selg[:], inv[:, 0:1],
                                                None, mybir.AluOpType.mult)
                        obuf = attp.tile([16, 56 * 56], F32, tag="obuf")
                        for ch in range(7):
                            pso = psco.tile([16, 448], F32, tag="pso")
                            nc.tensor.matmul(pso[:], sg2[:],
                                             att[:, ch * 448:(ch + 1) * 448],
                                             start=True, stop=True)
                            nc.vector.tensor_copy(
                                obuf[:, ch * 448:(ch + 1) * 448], pso[:])
                        nc.gpsimd.dma_start(
                            out.rearrange("i o a b -> (i o) (a b)"), obuf[:])

            if r_loop:
                with tc.For_i(0, r_loop, 1):
                    emit_body()
            else:
                emit_body()
    nc.compile()
    return nc


def prep_inputs(inputs, conv_dtype="fp8", mm_mode="swi"):
    """Host prep: fold BN/pool scales, quantize, build layouts, shard batch."""
    import ml_dtypes
    F8NP = ml_dtypes.float8_e4m3

    x = inputs["x"]
    eps_s = 1.0 / np.sqrt(np.float32(1.0 + 1e-5))

    def fold(w, g):
        s = (g * eps_s).astype(np.float32)
        return (w * s[:, None, None, None]).astype(np.float32)

    w1 = fold(inputs["w1"], inputs["g1"])            # [256,512,3,3]
    w2 = fold(inputs["w2"], inputs["g2"])            # [256,256,3,3]
    w3 = fold(inputs["w3"], inputs["g3"]) / 256.0    # avgpool16 norm
    w4 = fold(inputs["w4"], inputs["g4"])
    w5 = fold(inputs["w5"], inputs["g5"]) / 9.0      # avgpool3 norm
    wfc = np.asarray(inputs["w_fc"], np.float32)     # [128, 576]
    mw = np.asarray(inputs["mix_w"], np.float32).reshape(OUT, GMM)
    mw = np.exp(mw - mw.max(1, keepdims=True))
    mw = mw / mw.sum(1, keepdims=True)               # softmax over gmm

    w2q = w2.astype(F8NP).astype(np.float32)
    # conv2 fp8 mean-correction: corr[co] = S @ mean(h1), S = sum_taps dw2
    S = (w2 - w2q).sum(axis=(2, 3)) / float(H * W)   # [co, ci]
    selS = np.ascontiguousarray(
        S.reshape(2, 128, 2, 128).transpose(3, 2, 0, 1))  # [cip,cic,coc,cop]

    # conv weights -> [128(p=cin%128), 9(tap), ncin, cout]
    def wt_layout(w, ncin):
        co = w.shape[0]
        r = w.transpose(1, 2, 3, 0).reshape(ncin, 128, 9, co)
        return np.ascontiguousarray(r.transpose(1, 2, 0, 3))

    def swi_layout(wt, npair):
        # wt: [128, 9, ncin, co]; out: [128, 9, npair(, 2coc), 256] where the
        # last dim holds (A[127-j], B[127-j]) interleaved pairs per co chunk
        ncin, co = wt.shape[2], wt.shape[3]
        ncoc = co // 128
        w = wt.reshape(128, 9, npair, 2, ncoc, 128)       # [.., pair, b, coc, cow]
        w = w[..., ::-1]                                   # reverse cow
        w = w.transpose(0, 1, 2, 4, 5, 3)                  # [.., pair, coc, cow, b]
        out = w.reshape(128, 9, npair, ncoc, 256)
        if npair == 1:
            out = out[:, :, 0]
        return np.ascontiguousarray(out)

    w1f = wt_layout(w1.astype(F8NP).astype(np.float32), 4)
    w2f = wt_layout(w2q, 2)
    if mm_mode == "swi":
        w1t = swi_layout(w1f, 2).astype(F8NP)
        w2t = swi_layout(w2f, 1).astype(F8NP)
    else:
        w1t = w1f.astype(F8NP)
        w2t = w2f.astype(F8NP)
    w3t = wt_layout(w3, 2)
    w4t = wt_layout(w4, 1)[:, :, 0, :]
    w5t = wt_layout(w5, 1)[:, :, 0, :]
    wfct = np.ascontiguousarray(wfc.reshape(128, 64, 9).transpose(1, 2, 0))

    def bias_chunks(b, nchunk):
        return np.ascontiguousarray(
            np.asarray(b, np.float32).reshape(nchunk, 128).T)

    b1h = bias_chunks(inputs["b1"], 2)
    b2h = bias_chunks(inputs["b2"], 2)
    b3h = np.asarray(inputs["b3"], np.float32).reshape(128, 1)
    b4h = np.asarray(inputs["b4"], np.float32).reshape(128, 1)
    b5h = np.asarray(inputs["b5"], np.float32).reshape(64, 1)

    selp = np.zeros((128, 128), np.float32)
    for m in range(32):
        selp[4 * m + 0, m] = float(H2 - 1)
        selp[4 * m + 1, m + 32] = float(W2 - 1)
        selp[4 * m + 2, m + 64] = 2.0 * LOGR
        selp[4 * m + 3, m + 96] = 1.6
    selg = np.zeros((64, 16), np.float32)
    for i in range(IMG):
        for o in range(OUT):
            for g in range(GMM):
                selg[32 * i + o * GMM + g, 8 * i + o] = mw[o, g]
    negio = np.broadcast_to(-np.arange(56, dtype=np.float32), (64, 56)).copy()
    cst = np.full((64, 1), -LOGR, np.float32)

    xp = np.zeros((B, 4, 128, HP, WP), F8NP)
    xp[:, :, :, 1:113, 1:113] = np.asarray(x, np.float32).reshape(
        B, 4, 128, H, W).astype(F8NP)

    common = {
        "w1t": w1t, "w2t": w2t,
        "w3t": w3t, "w4t": w4t, "w5t": w5t, "wfct": wfct,
        "b1d": b1h, "b2d": b2h, "b3d": b3h, "b4d": b4h, "b5d": b5h,
        "selpd": selp, "selgd": selg, "negiod": negio, "cstd": cst,
        "selSd": selS,
    }
    in_maps = []
    for c in range(NCORE):
        m = dict(common)
        m["x"] = np.ascontiguousarray(xp[c * IMG:(c + 1) * IMG])
        in_maps.append(m)
    return in_maps


_CACHE = {}


def kernel(**inputs):
    inputs = {k: np.asarray(v) for k, v in inputs.items()}
    if "nc" not in _CACHE:
        _CACHE["nc"] = build_nc()
    nc = _CACHE["nc"]
    in_maps = prep_inputs(inputs)
    res = run_bass_kernel_spmd(nc, in_maps, core_ids=list(range(NCORE)))
    out = np.concatenate([res.results[c]["out"] for c in range(NCORE)], axis=0)
    return np.ascontiguousarray(out.astype(np.float32))
